# revision 18
# baseline (speedup 1.0000x reference)
"""MoE feed-forward (E=8 experts, top-2 routing) on 8 Trainium2 NeuronCores.

Strategy: expert-parallel dispatch. Host computes the (cheap, 0.07% of
FLOPs) routing exactly as the reference does, then packs the T*K=8192
(token, expert) pairs into an SPMD-uniform per-core structure of three
single-expert tiles with widths (W1, W2, W3) -- identical across cores,
expert binding and token content vary per core via the input tensors.
The widths are chosen by a small DP solver so that the 24 tiles (8 of
each width) cover the actual per-expert token counts with minimal total
capacity (1044 slots/core for the setup_inputs() routing vs 8192/8 =
1024 ideal), with gate-0 padding making the slack exact.

Device kernel (SPMD, same program all 8 cores): three passes, one tile
each; pass s streams expert weight set s from HBM per h-chunk:
    hT  = silu(Wg^T x + bg) * (W1^T x + b1)      [feature-major, [H, tok]]
    out = gate * (W2^T hT + b2)                  [[D, tok]]
All matmuls keep weights stationary / activations feature-major (no
transposes), bf16 inputs with fp32 PSUM accumulation.

Schedule (TimelineSim: 313.9us baseline -> 241.5us):
- pass order [widest, narrowest, middle]: the narrow pass's 17.3MB
  weight stream (vs only ~33us of PE work) is prefetched during the
  wide passes' PE surplus instead of stalling at the kernel tail;
- two HWDGE queues: weight streams own the SP queue, x/gate/bias/out
  DMAs go via the Activation queue, so the first matmuls wait on
  (x piece || first weight chunks), not their sum;
- phase 2 keeps the pass's W2 chunks SBUF-resident and computes output
  d-chunks in groups [4, 3, 1], so each group's bias/gate + out-DMA
  drain hides under the next group's matmuls (only 1 chunk exposed);
- fp8 was evaluated and rejected: e4m3 matmul chains measure 4-7%
  rel err vs the 2e-2 gate (bf16 chain: 0.41%).
"""

import numpy as np

E = 8
K = 2
D = 1024
H = 2736
B, S = 2, 2048
T = B * S
N_CORES = 8
P = 128
DC = D // P            # 8 d-chunks
HC = (H + P - 1) // P  # 22 h-chunks
HP = HC * P            # 2816 padded hidden
NSET = 3               # weight sets (= tiles) per core

_prog_cache: dict = {}


def _bf16(a):
    import ml_dtypes
    return np.ascontiguousarray(a.astype(ml_dtypes.bfloat16))


def _routing(x, centroid):
    """Mirror reference.py's routing math op-for-op (same platform => same
    top-k selection and softmax weights)."""
    import jax
    import jax.numpy as jnp
    xs = jnp.asarray(x, jnp.float32).reshape(T, D)
    c = jnp.asarray(centroid, jnp.float32)
    d2 = (jnp.sum(xs * xs, axis=-1, keepdims=True)
          + jnp.sum(c * c, axis=-1)[None, :]
          - 2.0 * (xs @ c.T))
    dist = jnp.sqrt(jnp.maximum(d2, 1e-12))
    w, sel = jax.lax.top_k(dist, K)
    w = jax.nn.softmax(w.astype(jnp.float32), axis=1)
    return np.asarray(sel), np.asarray(w, np.float32)


# Pre-solved dispatch for the deterministic setup_inputs() routing
# (avoids the ~1 min width search at run time; falls back to the solver
# for any other input).
_KNOWN_PLANS = {
    (725, 132, 1931, 1790, 1097, 470, 1266, 781): (
        (512, 384, 148),
        [(0, 2, 0), (0, 0, 1), (2, 2, 1), (2, 2, 0),
         (1, 0, 4), (1, 0, 0), (1, 2, 0), (1, 0, 2)],
    ),
}


def _solve_widths(cnt):
    """Find tile widths (512, w2, w3) s.t. the 8+8+8 single-expert tiles
    cover per-expert counts cnt, minimizing 512+w2+w3.  DP over experts:
    state (n 512-tiles used, n w2-tiles used) -> min w3-tiles used."""
    known = _KNOWN_PLANS.get(tuple(int(c) for c in cnt))
    if known is not None:
        return known
    def feas(w1, w2, w3):
        dp = {(0, 0): (0, [])}
        for e in range(E):
            ndp = {}
            for (sa, sc), (sb, asg) in dp.items():
                for a in range(0, 9 - sa):
                    for c in range(0, 9 - sc):
                        rem = cnt[e] - a * w1 - c * w2
                        b = 0 if rem <= 0 else -(-rem // w3)
                        if b > 8 - sb:
                            continue
                        k = (sa + a, sc + c)
                        v = sb + b
                        if k not in ndp or ndp[k][0] > v:
                            ndp[k] = (v, asg + [(a, c, b)])
            dp = ndp
            if not dp:
                return None
        best = min(dp.values(), key=lambda x: x[0], default=None)
        return best[1] if best else None

    best = None
    for w2 in range(512, 296, -4):
        for w3 in range(64, min(w2, 260) + 1, 4):
            cap = 512 + w2 + w3
            if best and cap >= best[0]:
                continue
            a = feas(512, w2, w3)
            if a:
                best = (cap, w2, w3, a)
    if best is None:
        a = feas(512, 512, 512)
        assert a is not None, f"dispatch does not fit structure: {cnt}"
        best = (1536, 512, 512, a)
    return (512, best[1], best[2]), best[3]


def _build_plan(sel, w):
    """Dispatch: solve widths, bind tiles to experts, fill with tokens.

    Returns (widths, sets, slot_tok, slot_gate, slot_exp): per core c,
    sets[c] = (e_w1, e_w2, e_w3); slots are the concat of the 3 tiles.
    """
    cnt = np.bincount(sel.ravel(), minlength=E)
    widths, asg = _solve_widths(list(cnt))

    tok_e, gate_e = {}, {}
    for e in range(E):
        tks, ks = np.nonzero(sel == e)
        tok_e[e] = tks
        gate_e[e] = w[tks, ks]

    # tile lists per width class: [(expert, start, len), ...] x8 each
    tiles = [[], [], []]
    for e in range(E):
        a_e = asg[e]
        pos = 0
        n_e = int(cnt[e])
        for cls in range(NSET):
            for _ in range(a_e[cls]):
                take = min(widths[cls], n_e - pos)
                take = max(take, 0)
                tiles[cls].append((e, pos, take))
                pos += take
        assert pos >= n_e, f"expert {e} tokens not covered"
    for cls in range(NSET):
        assert len(tiles[cls]) <= N_CORES
        while len(tiles[cls]) < N_CORES:
            tiles[cls].append((0, 0, 0))
        # biggest tiles first for stable packing
        tiles[cls].sort(key=lambda t: -t[2])

    sets, slot_tok, slot_gate, slot_exp = [], [], [], []
    for c in range(N_CORES):
        es, toks, gates, exps = [], [], [], []
        for cls in range(NSET):
            e, pos, take = tiles[cls][c]
            wd = widths[cls]
            t = np.zeros(wd, np.int64)
            g = np.zeros(wd, np.float32)
            t[:take] = tok_e[e][pos:pos + take]
            g[:take] = gate_e[e][pos:pos + take]
            es.append(e)
            toks.append(t)
            gates.append(g)
            exps.append(np.full(wd, e, np.int64))
        sets.append(tuple(es))
        slot_tok.append(np.concatenate(toks))
        slot_gate.append(np.concatenate(gates))
        slot_exp.append(np.concatenate(exps))
    return widths, sets, slot_tok, slot_gate, slot_exp


def _build_program(widths, reps=1, only_pass=None, ph2_dmajor=False,
                   passes=None, skip_phase2=False, skip_out=False):
    """Build + compile the SPMD Bass program for tile widths (w1,w2,w3).

    reps>1 replicates the FFN body (same inputs/outputs) for the
    launch-slope timing differential in test.py.  only_pass/passes/
    skip_phase2/skip_out are timing-experiment knobs; ph2_dmajor selects
    the phase-2 matmul issue order (d-major chains vs h-interleaved)."""
    import sys
    if "/opt/trn_rl_repo" not in sys.path:
        sys.path.insert(0, "/opt/trn_rl_repo")
    import concourse.bacc as bacc
    import concourse.bass as bass
    import concourse.tile as tile
    from concourse import mybir

    f32 = mybir.dt.float32
    bf16 = mybir.dt.bfloat16
    AF = mybir.ActivationFunctionType
    OP = mybir.AluOpType

    CAP = sum(widths)
    offs = [0, widths[0], widths[0] + widths[1]]

    nc = bacc.Bacc("TRN2", target_bir_lowering=False, num_devices=N_CORES)
    xt_ds = [nc.dram_tensor(f"xt{s}", [P, DC, widths[s]], bf16,
                            kind="ExternalInput") for s in range(NSET)]
    g_d = nc.dram_tensor("gates", [1, CAP], f32, kind="ExternalInput")
    wg_d = nc.dram_tensor("wg", [NSET, HC, P, DC, P], bf16,
                          kind="ExternalInput")
    w1_d = nc.dram_tensor("w1", [NSET, HC, P, DC, P], bf16,
                          kind="ExternalInput")
    w2_d = nc.dram_tensor("w2", [NSET, P, HC, DC, P], bf16,
                          kind="ExternalInput")
    bg_d = nc.dram_tensor("bg", [P, NSET * HC], f32, kind="ExternalInput")
    b1_d = nc.dram_tensor("b1", [P, NSET * HC], f32, kind="ExternalInput")
    b2_d = nc.dram_tensor("b2", [P, NSET * DC], f32, kind="ExternalInput")
    out_d = nc.dram_tensor("out", [DC, P, CAP], bf16, kind="ExternalOutput")

    with tile.TileContext(nc) as tc:
        with (
            tc.tile_pool(name="xp", bufs=1) as xp,
            tc.tile_pool(name="gp", bufs=1) as gp,
            tc.tile_pool(name="bp", bufs=1) as bp,
            tc.tile_pool(name="wgp", bufs=16) as wgp,
            tc.tile_pool(name="w1p", bufs=16) as w1p,
            tc.tile_pool(name="w2cp", bufs=HC) as w2cp,
            tc.tile_pool(name="hp", bufs=1) as hp,
            tc.tile_pool(name="sgp", bufs=3) as sgp,
            tc.tile_pool(name="op", bufs=4) as op_,
            tc.tile_pool(name="pp", bufs=8, space="PSUM") as pp,
        ):
            # Pass order: widest pass first (PE-rich start), narrowest in
            # the middle (its DMA-heavy weight stream is prefetched during
            # the first pass), medium last (its longer phase 2 hides the
            # output drain chain at the kernel tail).
            order = sorted(range(NSET), key=lambda s: -widths[s])
            order = [order[0], order[2], order[1]]

            # DMA queue split: the big expert-weight streams own the SP
            # HWDGE queue; activations, gate, biases and outputs go via
            # the Activation HWDGE queue so the first matmuls wait on
            # (xt piece || wg h0+w1 h0) instead of their sum.
            xt = xp.tile([P, DC, CAP], bf16)
            for i, s in enumerate(order):
                if i == 0:
                    # first pass's segment in three pieces: the d=0 matmuls
                    # start earlier, and the middle piece lands before the
                    # pg chain reaches d=2
                    nc.scalar.dma_start(
                        out=xt[:, 0:2, offs[s]:offs[s] + widths[s]],
                        in_=xt_ds[s][:, 0:2])
                    nc.scalar.dma_start(
                        out=xt[:, 2:5, offs[s]:offs[s] + widths[s]],
                        in_=xt_ds[s][:, 2:5])
                    nc.scalar.dma_start(
                        out=xt[:, 5:, offs[s]:offs[s] + widths[s]],
                        in_=xt_ds[s][:, 5:])
                else:
                    nc.scalar.dma_start(
                        out=xt[:, :, offs[s]:offs[s] + widths[s]],
                        in_=xt_ds[s][:])
            bg = bp.tile([P, NSET * HC], f32, tag="bg")
            b1 = bp.tile([P, NSET * HC], f32, tag="b1")
            b2 = bp.tile([P, NSET * DC], f32, tag="b2")
            nc.scalar.dma_start(out=bg[:], in_=bg_d[:])
            nc.scalar.dma_start(out=b1[:], in_=b1_d[:])
            nc.scalar.dma_start(out=b2[:], in_=b2_d[:])
            gate = gp.tile([P, CAP], f32)
            g_ap = g_d[:]
            nc.scalar.dma_start(
                out=gate[:],
                in_=bass.AP(tensor=g_ap.tensor, offset=g_ap.offset,
                            ap=[[0, P], [1, CAP]]))

            def ffn(s, w, off, ht, first=False):
                """One weight-set pass over a single w-wide token tile."""
                for h in range(HC):
                    wgt = wgp.tile([P, DC, P], bf16)
                    w1t = w1p.tile([P, DC, P], bf16)
                    nc.sync.dma_start(out=wgt[:], in_=wg_d[s, h])
                    nc.sync.dma_start(out=w1t[:], in_=w1_d[s, h])
                    pg = pp.tile([P, 512], f32, tag="ps", name="pg")
                    p1 = pp.tile([P, 512], f32, tag="ps", name="p1")
                    for d in range(DC):
                        nc.tensor.matmul(
                            pg[:, :w], wgt[:, d, :],
                            xt[:, d, off:off + w],
                            start=(d == 0), stop=(d == DC - 1))
                    for d in range(DC):
                        nc.tensor.matmul(
                            p1[:, :w], w1t[:, d, :],
                            xt[:, d, off:off + w],
                            start=(d == 0), stop=(d == DC - 1))
                    sg = sgp.tile([P, 512], f32)
                    nc.scalar.activation(
                        out=sg[:, :w], in_=pg[:, :w], func=AF.Silu,
                        bias=bg[:, s * HC + h:s * HC + h + 1], scale=1.0)
                    nc.vector.scalar_tensor_tensor(
                        out=ht[:, h, :w], in0=p1[:, :w],
                        scalar=b1[:, s * HC + h:s * HC + h + 1],
                        in1=sg[:, :w], op0=OP.add, op1=OP.mult)
                if skip_phase2:
                    return
                # phase 2: W2 chunks SBUF-resident (streamed once), output
                # d-chunks computed in groups [4, 3, 1] so the bias/gate
                # drain + out-DMA of group g overlaps group g+1's matmuls
                # instead of serializing after the last matmul.
                w2cs = []
                for h in range(HC):
                    w2c = w2cp.tile([P, DC, P], bf16)
                    nc.sync.dma_start(out=w2c[:], in_=w2_d[s, :, h])
                    w2cs.append(w2c)
                for ds_, de in ((0, 4), (4, 7), (7, 8)):
                    pos = [pp.tile([P, 512], f32, tag="ps", name="po")
                           for _ in range(de - ds_)]
                    if ph2_dmajor:
                        # d-major: long same-bank accumulation chains.
                        for i, d in enumerate(range(ds_, de)):
                            for h in range(HC):
                                nc.tensor.matmul(
                                    pos[i][:, :w], w2cs[h][:, d, :],
                                    ht[:, h, :w],
                                    start=(h == 0), stop=(h == HC - 1))
                    else:
                        for h in range(HC):
                            for i, d in enumerate(range(ds_, de)):
                                nc.tensor.matmul(
                                    pos[i][:, :w], w2cs[h][:, d, :],
                                    ht[:, h, :w],
                                    start=(h == 0), stop=(h == HC - 1))
                    for i, d in enumerate(range(ds_, de)):
                        osb = op_.tile([P, 512], bf16)
                        nc.vector.scalar_tensor_tensor(
                            out=osb[:, :w], in0=pos[i][:, :w],
                            scalar=b2[:, s * DC + d:s * DC + d + 1],
                            in1=gate[:, off:off + w],
                            op0=OP.add, op1=OP.mult)
                        if not skip_out:
                            nc.scalar.dma_start(out=out_d[d, :, off:off + w],
                                                in_=osb[:, :w])

            if passes is None:
                passes_ = order if only_pass is None else [only_pass]
            else:
                passes_ = passes
            for rep in range(reps):
                for i, s in enumerate(passes_):
                    ht = hp.tile([P, HC, widths[s]], bf16, tag=f"ht{s}",
                                 name=f"ht{s}")
                    ffn(s, widths[s], offs[s], ht,
                        first=(rep == 0 and i == 0))
    nc.compile()
    return nc


def _pack_core_inputs(widths, sets_c, slot_tok_c, slot_gate_c, xs,
                      wg_pe, w1_pe, w2_pe, bg_pe, b1_pe, b2_pe):
    CAP = sum(widths)
    x_slots = xs[slot_tok_c]                        # [CAP, D] f32
    xt = np.ascontiguousarray(
        x_slots.T.reshape(DC, P, CAP).transpose(1, 0, 2))  # [P, DC, CAP]
    xt = _bf16(xt)
    o = 0
    xsegs = {}
    for s, wd in enumerate(widths):
        xsegs[f"xt{s}"] = np.ascontiguousarray(xt[:, :, o:o + wd])
        o += wd
    return {
        **xsegs,
        "gates": slot_gate_c.reshape(1, CAP),
        "wg": np.stack([wg_pe[e] for e in sets_c]),
        "w1": np.stack([w1_pe[e] for e in sets_c]),
        "w2": np.stack([w2_pe[e] for e in sets_c]),
        "bg": np.concatenate([bg_pe[e] for e in sets_c], 1),
        "b1": np.concatenate([b1_pe[e] for e in sets_c], 1),
        "b2": np.concatenate([b2_pe[e] for e in sets_c], 1),
    }


def prepare(x, centroid, Wg, bg, W1, b1, W2, b2):
    """Host side: routing + dispatch. Returns (nc, in_maps, plan)."""
    x = np.asarray(x, np.float32)
    centroid = np.asarray(centroid, np.float32)
    Wg = np.asarray(Wg, np.float32)
    W1 = np.asarray(W1, np.float32)
    W2 = np.asarray(W2, np.float32)
    bg = np.asarray(bg, np.float32)
    b1 = np.asarray(b1, np.float32)
    b2 = np.asarray(b2, np.float32)

    sel, w = _routing(x, centroid)
    widths, sets, slot_tok, slot_gate, slot_exp = _build_plan(sel, w)

    key = ("v3", widths)
    if key not in _prog_cache:
        _prog_cache[key] = _build_program(widths)
    nc = _prog_cache[key]
    global _last_widths
    _last_widths = widths

    WgP = np.zeros((E, D, HP), np.float32)
    WgP[:, :, :H] = Wg
    W1P = np.zeros((E, D, HP), np.float32)
    W1P[:, :, :H] = W1
    W2P = np.zeros((E, HP, D), np.float32)
    W2P[:, :H, :] = W2
    bgP = np.zeros((E, HP), np.float32)
    bgP[:, :H] = bg
    b1P = np.zeros((E, HP), np.float32)
    b1P[:, :H] = b1
    # [h, p, d, c] layouts
    wg_pe = [_bf16(WgP[e].reshape(DC, P, HC, P).transpose(2, 1, 0, 3))
             for e in range(E)]
    w1_pe = [_bf16(W1P[e].reshape(DC, P, HC, P).transpose(2, 1, 0, 3))
             for e in range(E)]
    # [p, h, d, c] layout
    w2_pe = [_bf16(W2P[e].reshape(HC, P, DC, P).transpose(1, 0, 2, 3))
             for e in range(E)]
    bg_pe = [np.ascontiguousarray(bgP[e].reshape(HC, P).T) for e in range(E)]
    b1_pe = [np.ascontiguousarray(b1P[e].reshape(HC, P).T) for e in range(E)]
    b2_pe = [np.ascontiguousarray(b2[e].reshape(DC, P).T) for e in range(E)]

    xs = x.reshape(T, D)
    in_maps = [
        _pack_core_inputs(widths, sets[c], slot_tok[c], slot_gate[c], xs,
                          wg_pe, w1_pe, w2_pe, bg_pe, b1_pe, b2_pe)
        for c in range(N_CORES)
    ]
    plan = (slot_tok, slot_gate, slot_exp)
    return nc, in_maps, plan


def combine(results, plan):
    """Scatter-add per-core outputs back to the full [B, S, D] output."""
    slot_tok, slot_gate, slot_exp = plan
    out = np.zeros((T, D), np.float32)
    cap = len(slot_tok[0])
    for e in range(E):
        idxs, vals = [], []
        for c in range(N_CORES):
            ovals = results[c]["out"]  # [DC, P, CAP] f32
            m = (slot_exp[c] == e) & (slot_gate[c] != 0.0)
            if not m.any():
                continue
            sl = np.nonzero(m)[0]
            idxs.append(slot_tok[c][sl])
            ov = np.asarray(ovals).astype(np.float32)
            vals.append(ov.reshape(D, cap)[:, sl].T)  # [n, D]
        if not idxs:
            continue
        idx = np.concatenate(idxs)
        val = np.concatenate(vals)
        # token indices are unique within one expert
        out[idx] += val
    return out.reshape(B, S, D)


def kernel(x, centroid, Wg, bg, W1, b1, W2, b2):
    import sys
    if "/opt/trn_rl_repo" not in sys.path:
        sys.path.insert(0, "/opt/trn_rl_repo")
    from concourse.bass_utils import run_bass_kernel_spmd

    nc, in_maps, plan = prepare(x, centroid, Wg, bg, W1, b1, W2, b2)
    res = run_bass_kernel_spmd(nc, in_maps, list(range(N_CORES)))
    return combine(res.results, plan)



# revision 27
# speedup vs baseline: 1.0006x; 1.0006x over previous
"""MoE feed-forward (E=8 experts, top-2 routing) on 8 Trainium2 NeuronCores.

Strategy: expert-parallel dispatch. Host computes the (cheap, 0.07% of
FLOPs) routing exactly as the reference does, then packs the T*K=8192
(token, expert) pairs into an SPMD-uniform per-core structure of three
single-expert tiles with widths (W1, W2, W3) -- identical across cores,
expert binding and token content vary per core via the input tensors.
The widths are chosen by a small DP solver so that the 24 tiles (8 of
each width) cover the actual per-expert token counts with minimal total
capacity (1044 slots/core for the setup_inputs() routing vs 8192/8 =
1024 ideal), with gate-0 padding making the slack exact.

Device kernel (SPMD, same program all 8 cores): three passes, one tile
each; pass s streams expert weight set s from HBM per h-chunk:
    hT  = silu(Wg^T x + bg) * (W1^T x + b1)      [feature-major, [H, tok]]
    out = gate * (W2^T hT + b2)                  [[D, tok]]
All matmuls keep weights stationary / activations feature-major (no
transposes), bf16 inputs with fp32 PSUM accumulation.

Schedule (TimelineSim: 313.9us baseline -> 241.5us):
- pass order [widest, narrowest, middle]: the narrow pass's 17.3MB
  weight stream (vs only ~33us of PE work) is prefetched during the
  wide passes' PE surplus instead of stalling at the kernel tail;
- two HWDGE queues: weight streams own the SP queue, x/gate/bias/out
  DMAs go via the Activation queue, so the first matmuls wait on
  (x piece || first weight chunks), not their sum;
- phase 2 keeps the pass's W2 chunks SBUF-resident and computes output
  d-chunks in groups [4, 3, 1], so each group's bias/gate + out-DMA
  drain hides under the next group's matmuls (only 1 chunk exposed);
- fp8 was evaluated and rejected: e4m3 matmul chains measure 4-7%
  rel err vs the 2e-2 gate (bf16 chain: 0.41%).
"""

import numpy as np

E = 8
K = 2
D = 1024
H = 2736
B, S = 2, 2048
T = B * S
N_CORES = 8
P = 128
DC = D // P            # 8 d-chunks
HC = (H + P - 1) // P  # 22 h-chunks
HP = HC * P            # 2816 padded hidden
NSET = 3               # weight sets (= tiles) per core

_prog_cache: dict = {}


def _bf16(a):
    import ml_dtypes
    return np.ascontiguousarray(a.astype(ml_dtypes.bfloat16))


def _routing(x, centroid):
    """Mirror reference.py's routing math op-for-op (same platform => same
    top-k selection and softmax weights)."""
    import jax
    import jax.numpy as jnp
    xs = jnp.asarray(x, jnp.float32).reshape(T, D)
    c = jnp.asarray(centroid, jnp.float32)
    d2 = (jnp.sum(xs * xs, axis=-1, keepdims=True)
          + jnp.sum(c * c, axis=-1)[None, :]
          - 2.0 * (xs @ c.T))
    dist = jnp.sqrt(jnp.maximum(d2, 1e-12))
    w, sel = jax.lax.top_k(dist, K)
    w = jax.nn.softmax(w.astype(jnp.float32), axis=1)
    return np.asarray(sel), np.asarray(w, np.float32)


# Pre-solved dispatch for the deterministic setup_inputs() routing
# (avoids the ~1 min width search at run time; falls back to the solver
# for any other input).
_KNOWN_PLANS = {
    (725, 132, 1931, 1790, 1097, 470, 1266, 781): (
        (512, 384, 148),
        [(0, 2, 0), (0, 0, 1), (2, 2, 1), (2, 2, 0),
         (1, 0, 4), (1, 0, 0), (1, 2, 0), (1, 0, 2)],
    ),
}


def _solve_widths(cnt):
    """Find tile widths (512, w2, w3) s.t. the 8+8+8 single-expert tiles
    cover per-expert counts cnt, minimizing 512+w2+w3.  DP over experts:
    state (n 512-tiles used, n w2-tiles used) -> min w3-tiles used."""
    known = _KNOWN_PLANS.get(tuple(int(c) for c in cnt))
    if known is not None:
        return known
    def feas(w1, w2, w3):
        dp = {(0, 0): (0, [])}
        for e in range(E):
            ndp = {}
            for (sa, sc), (sb, asg) in dp.items():
                for a in range(0, 9 - sa):
                    for c in range(0, 9 - sc):
                        rem = cnt[e] - a * w1 - c * w2
                        b = 0 if rem <= 0 else -(-rem // w3)
                        if b > 8 - sb:
                            continue
                        k = (sa + a, sc + c)
                        v = sb + b
                        if k not in ndp or ndp[k][0] > v:
                            ndp[k] = (v, asg + [(a, c, b)])
            dp = ndp
            if not dp:
                return None
        best = min(dp.values(), key=lambda x: x[0], default=None)
        return best[1] if best else None

    best = None
    for w2 in range(512, 296, -4):
        for w3 in range(64, min(w2, 260) + 1, 4):
            cap = 512 + w2 + w3
            if best and cap >= best[0]:
                continue
            a = feas(512, w2, w3)
            if a:
                best = (cap, w2, w3, a)
    if best is None:
        a = feas(512, 512, 512)
        assert a is not None, f"dispatch does not fit structure: {cnt}"
        best = (1536, 512, 512, a)
    return (512, best[1], best[2]), best[3]


def _build_plan(sel, w):
    """Dispatch: solve widths, bind tiles to experts, fill with tokens.

    Returns (widths, sets, slot_tok, slot_gate, slot_exp): per core c,
    sets[c] = (e_w1, e_w2, e_w3); slots are the concat of the 3 tiles.
    """
    cnt = np.bincount(sel.ravel(), minlength=E)
    widths, asg = _solve_widths(list(cnt))

    tok_e, gate_e = {}, {}
    for e in range(E):
        tks, ks = np.nonzero(sel == e)
        tok_e[e] = tks
        gate_e[e] = w[tks, ks]

    # tile lists per width class: [(expert, start, len), ...] x8 each
    tiles = [[], [], []]
    for e in range(E):
        a_e = asg[e]
        pos = 0
        n_e = int(cnt[e])
        for cls in range(NSET):
            for _ in range(a_e[cls]):
                take = min(widths[cls], n_e - pos)
                take = max(take, 0)
                tiles[cls].append((e, pos, take))
                pos += take
        assert pos >= n_e, f"expert {e} tokens not covered"
    for cls in range(NSET):
        assert len(tiles[cls]) <= N_CORES
        while len(tiles[cls]) < N_CORES:
            tiles[cls].append((0, 0, 0))
        # biggest tiles first for stable packing
        tiles[cls].sort(key=lambda t: -t[2])

    sets, slot_tok, slot_gate, slot_exp = [], [], [], []
    for c in range(N_CORES):
        es, toks, gates, exps = [], [], [], []
        for cls in range(NSET):
            e, pos, take = tiles[cls][c]
            wd = widths[cls]
            t = np.zeros(wd, np.int64)
            g = np.zeros(wd, np.float32)
            t[:take] = tok_e[e][pos:pos + take]
            g[:take] = gate_e[e][pos:pos + take]
            es.append(e)
            toks.append(t)
            gates.append(g)
            exps.append(np.full(wd, e, np.int64))
        sets.append(tuple(es))
        slot_tok.append(np.concatenate(toks))
        slot_gate.append(np.concatenate(gates))
        slot_exp.append(np.concatenate(exps))
    return widths, sets, slot_tok, slot_gate, slot_exp


def _build_program(widths, reps=1, only_pass=None, ph2_dmajor=False,
                   passes=None, skip_phase2=False, skip_out=False):
    """Build + compile the SPMD Bass program for tile widths (w1,w2,w3).

    reps>1 replicates the FFN body (same inputs/outputs) for the
    launch-slope timing differential in test.py.  only_pass/passes/
    skip_phase2/skip_out are timing-experiment knobs; ph2_dmajor selects
    the phase-2 matmul issue order (d-major chains vs h-interleaved)."""
    import sys
    if "/opt/trn_rl_repo" not in sys.path:
        sys.path.insert(0, "/opt/trn_rl_repo")
    import concourse.bacc as bacc
    import concourse.bass as bass
    import concourse.tile as tile
    from concourse import mybir

    f32 = mybir.dt.float32
    bf16 = mybir.dt.bfloat16
    AF = mybir.ActivationFunctionType
    OP = mybir.AluOpType

    CAP = sum(widths)
    offs = [0, widths[0], widths[0] + widths[1]]

    nc = bacc.Bacc("TRN2", target_bir_lowering=False, num_devices=N_CORES)
    xt_ds = [nc.dram_tensor(f"xt{s}", [P, DC, widths[s]], bf16,
                            kind="ExternalInput") for s in range(NSET)]
    g_d = nc.dram_tensor("gates", [1, CAP], f32, kind="ExternalInput")
    wg_d = nc.dram_tensor("wg", [NSET, HC, P, DC, P], bf16,
                          kind="ExternalInput")
    w1_d = nc.dram_tensor("w1", [NSET, HC, P, DC, P], bf16,
                          kind="ExternalInput")
    w2_d = nc.dram_tensor("w2", [NSET, P, HC, DC, P], bf16,
                          kind="ExternalInput")
    bg_d = nc.dram_tensor("bg", [P, NSET * HC], f32, kind="ExternalInput")
    b1_d = nc.dram_tensor("b1", [P, NSET * HC], f32, kind="ExternalInput")
    b2_d = nc.dram_tensor("b2", [P, NSET * DC], f32, kind="ExternalInput")
    out_d = nc.dram_tensor("out", [DC, P, CAP], bf16, kind="ExternalOutput")

    with tile.TileContext(nc) as tc:
        with (
            tc.tile_pool(name="xp", bufs=1) as xp,
            tc.tile_pool(name="gp", bufs=1) as gp,
            tc.tile_pool(name="bp", bufs=1) as bp,
            tc.tile_pool(name="wgp", bufs=16) as wgp,
            tc.tile_pool(name="w1p", bufs=16) as w1p,
            tc.tile_pool(name="w2cp", bufs=HC) as w2cp,
            tc.tile_pool(name="hp", bufs=1) as hp,
            tc.tile_pool(name="sgp", bufs=3) as sgp,
            tc.tile_pool(name="op", bufs=4) as op_,
            tc.tile_pool(name="pp", bufs=8, space="PSUM") as pp,
        ):
            # Pass order: widest pass first (PE-rich start), narrowest in
            # the middle (its DMA-heavy weight stream is prefetched during
            # the first pass), medium last (its longer phase 2 hides the
            # output drain chain at the kernel tail).
            order = sorted(range(NSET), key=lambda s: -widths[s])
            order = [order[0], order[2], order[1]]

            # PE p-state warm-up: the cost of the 3us frequency ramp is paid
            # during the DMA-bound prologue (PE would idle anyway) instead
            # of during the first real matmul chains.
            wu = xp.tile([P, P], bf16, tag="wu")
            nc.vector.memset(wu[:], 0.0)
            pwu = pp.tile([P, 512], f32, tag="ps", name="pwu")
            for _k in range(12):
                nc.tensor.matmul(pwu[:, :P], wu[:], wu[:],
                                 start=True, stop=True)

            # DMA queue split: the big expert-weight streams own the SP
            # HWDGE queue; activations, gate, biases and outputs go via
            # the Activation HWDGE queue so the first matmuls wait on
            # (xt piece || wg h0+w1 h0) instead of their sum.
            xt = xp.tile([P, DC, CAP], bf16)
            for i, s in enumerate(order):
                if i == 0:
                    # first pass's segment in three pieces: the d=0 matmuls
                    # start earlier, and the middle piece lands before the
                    # pg chain reaches d=2
                    nc.scalar.dma_start(
                        out=xt[:, 0:2, offs[s]:offs[s] + widths[s]],
                        in_=xt_ds[s][:, 0:2])
                    nc.scalar.dma_start(
                        out=xt[:, 2:5, offs[s]:offs[s] + widths[s]],
                        in_=xt_ds[s][:, 2:5])
                    nc.scalar.dma_start(
                        out=xt[:, 5:, offs[s]:offs[s] + widths[s]],
                        in_=xt_ds[s][:, 5:])
                else:
                    nc.scalar.dma_start(
                        out=xt[:, :, offs[s]:offs[s] + widths[s]],
                        in_=xt_ds[s][:])
            bg = bp.tile([P, NSET * HC], f32, tag="bg")
            b1 = bp.tile([P, NSET * HC], f32, tag="b1")
            b2 = bp.tile([P, NSET * DC], f32, tag="b2")
            nc.scalar.dma_start(out=bg[:], in_=bg_d[:])
            nc.scalar.dma_start(out=b1[:], in_=b1_d[:])
            nc.scalar.dma_start(out=b2[:], in_=b2_d[:])
            gate = gp.tile([P, CAP], f32)
            g_ap = g_d[:]
            nc.scalar.dma_start(
                out=gate[:],
                in_=bass.AP(tensor=g_ap.tensor, offset=g_ap.offset,
                            ap=[[0, P], [1, CAP]]))

            def ffn(s, w, off, ht, first=False):
                """One weight-set pass over a single w-wide token tile."""
                for h in range(HC):
                    wgt = wgp.tile([P, DC, P], bf16)
                    w1t = w1p.tile([P, DC, P], bf16)
                    nc.sync.dma_start(out=wgt[:], in_=wg_d[s, h])
                    nc.sync.dma_start(out=w1t[:], in_=w1_d[s, h])
                    pg = pp.tile([P, 512], f32, tag="ps", name="pg")
                    p1 = pp.tile([P, 512], f32, tag="ps", name="p1")
                    for d in range(DC):
                        nc.tensor.matmul(
                            pg[:, :w], wgt[:, d, :],
                            xt[:, d, off:off + w],
                            start=(d == 0), stop=(d == DC - 1))
                    for d in range(DC):
                        nc.tensor.matmul(
                            p1[:, :w], w1t[:, d, :],
                            xt[:, d, off:off + w],
                            start=(d == 0), stop=(d == DC - 1))
                    sg = sgp.tile([P, 512], f32)
                    nc.scalar.activation(
                        out=sg[:, :w], in_=pg[:, :w], func=AF.Silu,
                        bias=bg[:, s * HC + h:s * HC + h + 1], scale=1.0)
                    nc.vector.scalar_tensor_tensor(
                        out=ht[:, h, :w], in0=p1[:, :w],
                        scalar=b1[:, s * HC + h:s * HC + h + 1],
                        in1=sg[:, :w], op0=OP.add, op1=OP.mult)
                if skip_phase2:
                    return
                # phase 2: W2 chunks SBUF-resident (streamed once), output
                # d-chunks computed in groups [4, 3, 1] so the bias/gate
                # drain + out-DMA of group g overlaps group g+1's matmuls
                # instead of serializing after the last matmul.
                w2cs = []
                for h in range(HC):
                    w2c = w2cp.tile([P, DC, P], bf16)
                    nc.sync.dma_start(out=w2c[:], in_=w2_d[s, :, h])
                    w2cs.append(w2c)
                # d=7 is split into two column-halves processed as separate
                # mini-groups: the first half's stt+out-DMA drain hides
                # under the second half's matmuls, so only a half-width
                # drain chain trails the kernel's last matmul.
                wh = (w + 1) // 2
                groups = [(0, 4, 0, w), (4, 7, 0, w),
                          (7, 8, 0, wh), (7, 8, wh, w)]
                for ds_, de, c0, c1 in groups:
                    wg_ = c1 - c0
                    pos = [pp.tile([P, 512], f32, tag="ps", name="po")
                           for _ in range(de - ds_)]
                    if ph2_dmajor:
                        # d-major: long same-bank accumulation chains.
                        for i, d in enumerate(range(ds_, de)):
                            for h in range(HC):
                                nc.tensor.matmul(
                                    pos[i][:, :wg_], w2cs[h][:, d, :],
                                    ht[:, h, c0:c1],
                                    start=(h == 0), stop=(h == HC - 1))
                    else:
                        for h in range(HC):
                            for i, d in enumerate(range(ds_, de)):
                                nc.tensor.matmul(
                                    pos[i][:, :wg_], w2cs[h][:, d, :],
                                    ht[:, h, c0:c1],
                                    start=(h == 0), stop=(h == HC - 1))
                    for i, d in enumerate(range(ds_, de)):
                        osb = op_.tile([P, 512], bf16)
                        nc.vector.scalar_tensor_tensor(
                            out=osb[:, :wg_], in0=pos[i][:, :wg_],
                            scalar=b2[:, s * DC + d:s * DC + d + 1],
                            in1=gate[:, off + c0:off + c1],
                            op0=OP.add, op1=OP.mult)
                        if not skip_out:
                            # the trailing mini-groups' outs go via the SP
                            # queue (idle by then) so the kernel tail does
                            # not queue behind the previous group's drains
                            eng = nc.sync if de - ds_ == 1 else nc.scalar
                            eng.dma_start(
                                out=out_d[d, :, off + c0:off + c1],
                                in_=osb[:, :wg_])

            if passes is None:
                passes_ = order if only_pass is None else [only_pass]
            else:
                passes_ = passes
            for rep in range(reps):
                for i, s in enumerate(passes_):
                    ht = hp.tile([P, HC, widths[s]], bf16, tag=f"ht{s}",
                                 name=f"ht{s}")
                    ffn(s, widths[s], offs[s], ht,
                        first=(rep == 0 and i == 0))
    nc.compile()
    return nc


def _pack_core_inputs(widths, sets_c, slot_tok_c, slot_gate_c, xs,
                      wg_pe, w1_pe, w2_pe, bg_pe, b1_pe, b2_pe):
    CAP = sum(widths)
    x_slots = xs[slot_tok_c]                        # [CAP, D] f32
    xt = np.ascontiguousarray(
        x_slots.T.reshape(DC, P, CAP).transpose(1, 0, 2))  # [P, DC, CAP]
    xt = _bf16(xt)
    o = 0
    xsegs = {}
    for s, wd in enumerate(widths):
        xsegs[f"xt{s}"] = np.ascontiguousarray(xt[:, :, o:o + wd])
        o += wd
    return {
        **xsegs,
        "gates": slot_gate_c.reshape(1, CAP),
        "wg": np.stack([wg_pe[e] for e in sets_c]),
        "w1": np.stack([w1_pe[e] for e in sets_c]),
        "w2": np.stack([w2_pe[e] for e in sets_c]),
        "bg": np.concatenate([bg_pe[e] for e in sets_c], 1),
        "b1": np.concatenate([b1_pe[e] for e in sets_c], 1),
        "b2": np.concatenate([b2_pe[e] for e in sets_c], 1),
    }


def prepare(x, centroid, Wg, bg, W1, b1, W2, b2):
    """Host side: routing + dispatch. Returns (nc, in_maps, plan)."""
    x = np.asarray(x, np.float32)
    centroid = np.asarray(centroid, np.float32)
    Wg = np.asarray(Wg, np.float32)
    W1 = np.asarray(W1, np.float32)
    W2 = np.asarray(W2, np.float32)
    bg = np.asarray(bg, np.float32)
    b1 = np.asarray(b1, np.float32)
    b2 = np.asarray(b2, np.float32)

    sel, w = _routing(x, centroid)
    widths, sets, slot_tok, slot_gate, slot_exp = _build_plan(sel, w)

    key = ("v3", widths)
    if key not in _prog_cache:
        _prog_cache[key] = _build_program(widths)
    nc = _prog_cache[key]
    global _last_widths
    _last_widths = widths

    WgP = np.zeros((E, D, HP), np.float32)
    WgP[:, :, :H] = Wg
    W1P = np.zeros((E, D, HP), np.float32)
    W1P[:, :, :H] = W1
    W2P = np.zeros((E, HP, D), np.float32)
    W2P[:, :H, :] = W2
    bgP = np.zeros((E, HP), np.float32)
    bgP[:, :H] = bg
    b1P = np.zeros((E, HP), np.float32)
    b1P[:, :H] = b1
    # [h, p, d, c] layouts
    wg_pe = [_bf16(WgP[e].reshape(DC, P, HC, P).transpose(2, 1, 0, 3))
             for e in range(E)]
    w1_pe = [_bf16(W1P[e].reshape(DC, P, HC, P).transpose(2, 1, 0, 3))
             for e in range(E)]
    # [p, h, d, c] layout
    w2_pe = [_bf16(W2P[e].reshape(HC, P, DC, P).transpose(1, 0, 2, 3))
             for e in range(E)]
    bg_pe = [np.ascontiguousarray(bgP[e].reshape(HC, P).T) for e in range(E)]
    b1_pe = [np.ascontiguousarray(b1P[e].reshape(HC, P).T) for e in range(E)]
    b2_pe = [np.ascontiguousarray(b2[e].reshape(DC, P).T) for e in range(E)]

    xs = x.reshape(T, D)
    in_maps = [
        _pack_core_inputs(widths, sets[c], slot_tok[c], slot_gate[c], xs,
                          wg_pe, w1_pe, w2_pe, bg_pe, b1_pe, b2_pe)
        for c in range(N_CORES)
    ]
    plan = (slot_tok, slot_gate, slot_exp)
    return nc, in_maps, plan


def combine(results, plan):
    """Scatter-add per-core outputs back to the full [B, S, D] output."""
    slot_tok, slot_gate, slot_exp = plan
    out = np.zeros((T, D), np.float32)
    cap = len(slot_tok[0])
    for e in range(E):
        idxs, vals = [], []
        for c in range(N_CORES):
            ovals = results[c]["out"]  # [DC, P, CAP] f32
            m = (slot_exp[c] == e) & (slot_gate[c] != 0.0)
            if not m.any():
                continue
            sl = np.nonzero(m)[0]
            idxs.append(slot_tok[c][sl])
            ov = np.asarray(ovals).astype(np.float32)
            vals.append(ov.reshape(D, cap)[:, sl].T)  # [n, D]
        if not idxs:
            continue
        idx = np.concatenate(idxs)
        val = np.concatenate(vals)
        # token indices are unique within one expert
        out[idx] += val
    return out.reshape(B, S, D)


def kernel(x, centroid, Wg, bg, W1, b1, W2, b2):
    import sys
    if "/opt/trn_rl_repo" not in sys.path:
        sys.path.insert(0, "/opt/trn_rl_repo")
    from concourse.bass_utils import run_bass_kernel_spmd

    nc, in_maps, plan = prepare(x, centroid, Wg, bg, W1, b1, W2, b2)
    res = run_bass_kernel_spmd(nc, in_maps, list(range(N_CORES)))
    return combine(res.results, plan)



# revision 30
# speedup vs baseline: 1.1105x; 1.1099x over previous
"""MoE feed-forward (E=8 experts, top-2 routing) on 8 Trainium2 NeuronCores.

Strategy: expert-parallel dispatch. Host computes the (cheap, 0.07% of
FLOPs) routing exactly as the reference does, then packs the T*K=8192
(token, expert) pairs into an SPMD-uniform per-core structure of three
single-expert tiles with widths (W1, W2, W3) -- identical across cores,
expert binding and token content vary per core via the input tensors.
The widths are chosen by a small DP solver so that the 24 tiles (8 of
each width) cover the actual per-expert token counts with minimal total
capacity (1044 slots/core for the setup_inputs() routing vs 8192/8 =
1024 ideal), with gate-0 padding making the slack exact.

Device kernel (SPMD, same program all 8 cores): three passes, one tile
each; pass s streams expert weight set s from HBM per h-chunk:
    hT  = silu(Wg^T x + bg) * (W1^T x + b1)      [feature-major, [H, tok]]
    out = gate * (W2^T hT + b2)                  [[D, tok]]
All matmuls keep weights stationary / activations feature-major (no
transposes), bf16 inputs with fp32 PSUM accumulation.

Schedule (TimelineSim: 313.9us baseline -> 241.5us -> 239.4us; the
steady state is exactly PE-bound at the bf16 roofline, 229.7us =
528*CAP cycles @2.4GHz, so only launch-boundary time is optimizable):
- pass order [widest, narrowest, middle]: the narrow pass's 17.3MB
  weight stream (vs only ~33us of PE work) is prefetched during the
  wide passes' PE surplus instead of stalling at the kernel tail;
- two HWDGE queues: weight streams own the SP queue, x/gate/bias/out
  DMAs go via the Activation queue, so the first matmuls wait on
  (x piece || first weight chunks), not their sum;
- 12 zero-operand warm-up matmuls during the DMA-bound prologue pay
  the PE p-state ramp (0.65->2.4GHz over 3us) while PE would idle
  anyway (-1.6us);
- first pass's x segment lands in 3 pieces sized to the pg chain's
  d-consumption; finer splits LOSE: each dma_start costs ~625ns on the
  serialized HWDGE issue pipe and transfers serialize at ~360GB/s;
- phase 2 keeps the pass's W2 chunks SBUF-resident and computes output
  d-chunks in groups [4, 3, 1+split]: the last d-chunk is two column
  halves so only a half-width stt+DMA chain trails the last matmul;
- outputs are written bf16 (rel err 4.43e-3 total vs the 2e-2 gate;
  f32 chain was 4.11e-3), halving out traffic;
- fp8 was evaluated and rejected: e4m3 matmul chains measure 4-7%
  rel err vs the 2e-2 gate; hi-lo fp8 decompositions cost >= bf16
  cycles on the DoubleRow path, so bf16 is the precision floor.

Measurement notes (axon pool): per-rep steady time must be fitted as
slope(reps_hi)-slope(reps_lo) with BOTH bodies > ~1ms (the per-launch
host feed is ~0.9ms and hides smaller bodies).  Sustained real-data
reps bodies power-throttle the PE ~1.3-1.7x within seconds (zero-data
bodies and short bursts do not), so single-launch grades track
TimelineSim (graded baseline 334.3us ~= 313.9us sim * 1.065), while
long real-data HW fits overestimate; validate schedule changes in sim
and with zero-input runs.
"""

import numpy as np

E = 8
K = 2
D = 1024
H = 2736
B, S = 2, 2048
T = B * S
N_CORES = 8
P = 128
DC = D // P            # 8 d-chunks
HC = (H + P - 1) // P  # 22 h-chunks
HP = HC * P            # 2816 padded hidden
NSET = 3               # weight sets (= tiles) per core

_prog_cache: dict = {}


def _bf16(a):
    import ml_dtypes
    return np.ascontiguousarray(a.astype(ml_dtypes.bfloat16))


def _routing(x, centroid):
    """Mirror reference.py's routing math op-for-op (same platform => same
    top-k selection and softmax weights)."""
    import jax
    import jax.numpy as jnp
    xs = jnp.asarray(x, jnp.float32).reshape(T, D)
    c = jnp.asarray(centroid, jnp.float32)
    d2 = (jnp.sum(xs * xs, axis=-1, keepdims=True)
          + jnp.sum(c * c, axis=-1)[None, :]
          - 2.0 * (xs @ c.T))
    dist = jnp.sqrt(jnp.maximum(d2, 1e-12))
    w, sel = jax.lax.top_k(dist, K)
    w = jax.nn.softmax(w.astype(jnp.float32), axis=1)
    return np.asarray(sel), np.asarray(w, np.float32)


# Pre-solved dispatch for the deterministic setup_inputs() routing
# (avoids the ~1 min width search at run time; falls back to the solver
# for any other input).
_KNOWN_PLANS = {
    (725, 132, 1931, 1790, 1097, 470, 1266, 781): (
        (512, 384, 148),
        [(0, 2, 0), (0, 0, 1), (2, 2, 1), (2, 2, 0),
         (1, 0, 4), (1, 0, 0), (1, 2, 0), (1, 0, 2)],
    ),
}


def _solve_widths(cnt):
    """Find tile widths (512, w2, w3) s.t. the 8+8+8 single-expert tiles
    cover per-expert counts cnt, minimizing 512+w2+w3.  DP over experts:
    state (n 512-tiles used, n w2-tiles used) -> min w3-tiles used."""
    known = _KNOWN_PLANS.get(tuple(int(c) for c in cnt))
    if known is not None:
        return known
    def feas(w1, w2, w3):
        dp = {(0, 0): (0, [])}
        for e in range(E):
            ndp = {}
            for (sa, sc), (sb, asg) in dp.items():
                for a in range(0, 9 - sa):
                    for c in range(0, 9 - sc):
                        rem = cnt[e] - a * w1 - c * w2
                        b = 0 if rem <= 0 else -(-rem // w3)
                        if b > 8 - sb:
                            continue
                        k = (sa + a, sc + c)
                        v = sb + b
                        if k not in ndp or ndp[k][0] > v:
                            ndp[k] = (v, asg + [(a, c, b)])
            dp = ndp
            if not dp:
                return None
        best = min(dp.values(), key=lambda x: x[0], default=None)
        return best[1] if best else None

    best = None
    for w2 in range(512, 296, -4):
        for w3 in range(64, min(w2, 260) + 1, 4):
            cap = 512 + w2 + w3
            if best and cap >= best[0]:
                continue
            a = feas(512, w2, w3)
            if a:
                best = (cap, w2, w3, a)
    if best is None:
        a = feas(512, 512, 512)
        assert a is not None, f"dispatch does not fit structure: {cnt}"
        best = (1536, 512, 512, a)
    return (512, best[1], best[2]), best[3]


def _build_plan(sel, w):
    """Dispatch: solve widths, bind tiles to experts, fill with tokens.

    Returns (widths, sets, slot_tok, slot_gate, slot_exp): per core c,
    sets[c] = (e_w1, e_w2, e_w3); slots are the concat of the 3 tiles.
    """
    cnt = np.bincount(sel.ravel(), minlength=E)
    widths, asg = _solve_widths(list(cnt))

    tok_e, gate_e = {}, {}
    for e in range(E):
        tks, ks = np.nonzero(sel == e)
        tok_e[e] = tks
        gate_e[e] = w[tks, ks]

    # tile lists per width class: [(expert, start, len), ...] x8 each
    tiles = [[], [], []]
    for e in range(E):
        a_e = asg[e]
        pos = 0
        n_e = int(cnt[e])
        for cls in range(NSET):
            for _ in range(a_e[cls]):
                take = min(widths[cls], n_e - pos)
                take = max(take, 0)
                tiles[cls].append((e, pos, take))
                pos += take
        assert pos >= n_e, f"expert {e} tokens not covered"
    for cls in range(NSET):
        assert len(tiles[cls]) <= N_CORES
        while len(tiles[cls]) < N_CORES:
            tiles[cls].append((0, 0, 0))
        # biggest tiles first for stable packing
        tiles[cls].sort(key=lambda t: -t[2])

    sets, slot_tok, slot_gate, slot_exp = [], [], [], []
    for c in range(N_CORES):
        es, toks, gates, exps = [], [], [], []
        for cls in range(NSET):
            e, pos, take = tiles[cls][c]
            wd = widths[cls]
            t = np.zeros(wd, np.int64)
            g = np.zeros(wd, np.float32)
            t[:take] = tok_e[e][pos:pos + take]
            g[:take] = gate_e[e][pos:pos + take]
            es.append(e)
            toks.append(t)
            gates.append(g)
            exps.append(np.full(wd, e, np.int64))
        sets.append(tuple(es))
        slot_tok.append(np.concatenate(toks))
        slot_gate.append(np.concatenate(gates))
        slot_exp.append(np.concatenate(exps))
    return widths, sets, slot_tok, slot_gate, slot_exp


def _build_program(widths, reps=1, only_pass=None, ph2_dmajor=False,
                   passes=None, skip_phase2=False, skip_out=False):
    """Build + compile the SPMD Bass program for tile widths (w1,w2,w3).

    reps>1 replicates the FFN body (same inputs/outputs) for the
    launch-slope timing differential in test.py.  only_pass/passes/
    skip_phase2/skip_out are timing-experiment knobs; ph2_dmajor selects
    the phase-2 matmul issue order (d-major chains vs h-interleaved)."""
    import sys
    if "/opt/trn_rl_repo" not in sys.path:
        sys.path.insert(0, "/opt/trn_rl_repo")
    import concourse.bacc as bacc
    import concourse.bass as bass
    import concourse.tile as tile
    from concourse import mybir

    f32 = mybir.dt.float32
    bf16 = mybir.dt.bfloat16
    AF = mybir.ActivationFunctionType
    OP = mybir.AluOpType

    CAP = sum(widths)
    offs = [0, widths[0], widths[0] + widths[1]]

    nc = bacc.Bacc("TRN2", target_bir_lowering=False, num_devices=N_CORES)
    xt_ds = [nc.dram_tensor(f"xt{s}", [P, DC, widths[s]], bf16,
                            kind="ExternalInput") for s in range(NSET)]
    g_d = nc.dram_tensor("gates", [1, CAP], f32, kind="ExternalInput")
    wg_d = nc.dram_tensor("wg", [NSET, HC, P, DC, P], bf16,
                          kind="ExternalInput")
    w1_d = nc.dram_tensor("w1", [NSET, HC, P, DC, P], bf16,
                          kind="ExternalInput")
    w2_d = nc.dram_tensor("w2", [NSET, P, HC, DC, P], bf16,
                          kind="ExternalInput")
    bg_d = nc.dram_tensor("bg", [P, NSET * HC], f32, kind="ExternalInput")
    b1_d = nc.dram_tensor("b1", [P, NSET * HC], f32, kind="ExternalInput")
    b2_d = nc.dram_tensor("b2", [P, NSET * DC], f32, kind="ExternalInput")
    out_d = nc.dram_tensor("out", [DC, P, CAP], bf16, kind="ExternalOutput")

    with tile.TileContext(nc) as tc:
        with (
            tc.tile_pool(name="xp", bufs=1) as xp,
            tc.tile_pool(name="gp", bufs=1) as gp,
            tc.tile_pool(name="bp", bufs=1) as bp,
            tc.tile_pool(name="wgp", bufs=16) as wgp,
            tc.tile_pool(name="w1p", bufs=16) as w1p,
            tc.tile_pool(name="w2cp", bufs=HC) as w2cp,
            tc.tile_pool(name="hp", bufs=1) as hp,
            tc.tile_pool(name="sgp", bufs=3) as sgp,
            tc.tile_pool(name="op", bufs=4) as op_,
            tc.tile_pool(name="pp", bufs=8, space="PSUM") as pp,
        ):
            # Pass order: widest pass first (PE-rich start), narrowest in
            # the middle (its DMA-heavy weight stream is prefetched during
            # the first pass), medium last (its longer phase 2 hides the
            # output drain chain at the kernel tail).
            order = sorted(range(NSET), key=lambda s: -widths[s])
            order = [order[0], order[2], order[1]]

            # PE p-state warm-up: the cost of the 3us frequency ramp is paid
            # during the DMA-bound prologue (PE would idle anyway) instead
            # of during the first real matmul chains.
            wu = xp.tile([P, P], bf16, tag="wu")
            nc.vector.memset(wu[:], 0.0)
            pwu = pp.tile([P, 512], f32, tag="ps", name="pwu")
            for _k in range(12):
                nc.tensor.matmul(pwu[:, :P], wu[:], wu[:],
                                 start=True, stop=True)

            # DMA queue split: the big expert-weight streams own the SP
            # HWDGE queue; activations, gate, biases and outputs go via
            # the Activation HWDGE queue so the first matmuls wait on
            # (xt piece || wg h0+w1 h0) instead of their sum.
            # prologue loads: only what the first pass pipeline needs soon.
            # The other passes' x segments, b2 and the 534KB gate broadcast
            # are deferred into the first pass (late_loads) so they don't
            # clog the serialized transfer pipe while the first weight
            # chunks are landing.
            xt = xp.tile([P, DC, CAP], bf16)
            s0 = order[0]
            # first pass's segment in three pieces: the d=0 matmuls start
            # earlier, and the middle piece lands before the pg chain
            # reaches d=2
            nc.scalar.dma_start(
                out=xt[:, 0:2, offs[s0]:offs[s0] + widths[s0]],
                in_=xt_ds[s0][:, 0:2])
            nc.scalar.dma_start(
                out=xt[:, 2:5, offs[s0]:offs[s0] + widths[s0]],
                in_=xt_ds[s0][:, 2:5])
            nc.scalar.dma_start(
                out=xt[:, 5:, offs[s0]:offs[s0] + widths[s0]],
                in_=xt_ds[s0][:, 5:])
            bg = bp.tile([P, NSET * HC], f32, tag="bg")
            b1 = bp.tile([P, NSET * HC], f32, tag="b1")
            b2 = bp.tile([P, NSET * DC], f32, tag="b2")
            nc.scalar.dma_start(out=bg[:], in_=bg_d[:])
            nc.scalar.dma_start(out=b1[:], in_=b1_d[:])
            gate = gp.tile([P, CAP], f32)

            def late_loads():
                for s in order[1:]:
                    nc.scalar.dma_start(
                        out=xt[:, :, offs[s]:offs[s] + widths[s]],
                        in_=xt_ds[s][:])
                nc.scalar.dma_start(out=b2[:], in_=b2_d[:])
                g_ap = g_d[:]
                nc.scalar.dma_start(
                    out=gate[:],
                    in_=bass.AP(tensor=g_ap.tensor, offset=g_ap.offset,
                                ap=[[0, P], [1, CAP]]))

            def ffn(s, w, off, ht, first=False):
                """One weight-set pass over a single w-wide token tile."""
                for h in range(HC):
                    wgt = wgp.tile([P, DC, P], bf16)
                    w1t = w1p.tile([P, DC, P], bf16)
                    nc.sync.dma_start(out=wgt[:], in_=wg_d[s, h])
                    nc.sync.dma_start(out=w1t[:], in_=w1_d[s, h])
                    if first and h == 8:
                        late_loads()
                    pg = pp.tile([P, 512], f32, tag="ps", name="pg")
                    p1 = pp.tile([P, 512], f32, tag="ps", name="p1")
                    for d in range(DC):
                        nc.tensor.matmul(
                            pg[:, :w], wgt[:, d, :],
                            xt[:, d, off:off + w],
                            start=(d == 0), stop=(d == DC - 1))
                    for d in range(DC):
                        nc.tensor.matmul(
                            p1[:, :w], w1t[:, d, :],
                            xt[:, d, off:off + w],
                            start=(d == 0), stop=(d == DC - 1))
                    sg = sgp.tile([P, 512], f32)
                    nc.scalar.activation(
                        out=sg[:, :w], in_=pg[:, :w], func=AF.Silu,
                        bias=bg[:, s * HC + h:s * HC + h + 1], scale=1.0)
                    nc.vector.scalar_tensor_tensor(
                        out=ht[:, h, :w], in0=p1[:, :w],
                        scalar=b1[:, s * HC + h:s * HC + h + 1],
                        in1=sg[:, :w], op0=OP.add, op1=OP.mult)
                if skip_phase2:
                    return
                # phase 2: W2 chunks SBUF-resident (streamed once), output
                # d-chunks computed in groups [4, 3, 1] so the bias/gate
                # drain + out-DMA of group g overlaps group g+1's matmuls
                # instead of serializing after the last matmul.
                w2cs = []
                for h in range(HC):
                    w2c = w2cp.tile([P, DC, P], bf16)
                    nc.sync.dma_start(out=w2c[:], in_=w2_d[s, :, h])
                    w2cs.append(w2c)
                # d=7 is split into two column-halves processed as separate
                # mini-groups: the first half's stt+out-DMA drain hides
                # under the second half's matmuls, so only a half-width
                # drain chain trails the kernel's last matmul.
                wh = (w + 1) // 2
                groups = [(0, 4, 0, w), (4, 7, 0, w),
                          (7, 8, 0, wh), (7, 8, wh, w)]
                for ds_, de, c0, c1 in groups:
                    wg_ = c1 - c0
                    pos = [pp.tile([P, 512], f32, tag="ps", name="po")
                           for _ in range(de - ds_)]
                    if ph2_dmajor:
                        # d-major: long same-bank accumulation chains.
                        for i, d in enumerate(range(ds_, de)):
                            for h in range(HC):
                                nc.tensor.matmul(
                                    pos[i][:, :wg_], w2cs[h][:, d, :],
                                    ht[:, h, c0:c1],
                                    start=(h == 0), stop=(h == HC - 1))
                    else:
                        for h in range(HC):
                            for i, d in enumerate(range(ds_, de)):
                                nc.tensor.matmul(
                                    pos[i][:, :wg_], w2cs[h][:, d, :],
                                    ht[:, h, c0:c1],
                                    start=(h == 0), stop=(h == HC - 1))
                    for i, d in enumerate(range(ds_, de)):
                        osb = op_.tile([P, 512], bf16)
                        nc.vector.scalar_tensor_tensor(
                            out=osb[:, :wg_], in0=pos[i][:, :wg_],
                            scalar=b2[:, s * DC + d:s * DC + d + 1],
                            in1=gate[:, off + c0:off + c1],
                            op0=OP.add, op1=OP.mult)
                        if not skip_out:
                            # the trailing mini-groups' outs go via the SP
                            # queue (idle by then) so the kernel tail does
                            # not queue behind the previous group's drains
                            eng = nc.sync if de - ds_ == 1 else nc.scalar
                            eng.dma_start(
                                out=out_d[d, :, off + c0:off + c1],
                                in_=osb[:, :wg_])

            if passes is None:
                passes_ = order if only_pass is None else [only_pass]
            else:
                passes_ = passes
            for rep in range(reps):
                for i, s in enumerate(passes_):
                    ht = hp.tile([P, HC, widths[s]], bf16, tag=f"ht{s}",
                                 name=f"ht{s}")
                    ffn(s, widths[s], offs[s], ht,
                        first=(rep == 0 and i == 0))
    nc.compile()
    return nc


def _pack_core_inputs(widths, sets_c, slot_tok_c, slot_gate_c, xs,
                      wg_pe, w1_pe, w2_pe, bg_pe, b1_pe, b2_pe):
    CAP = sum(widths)
    x_slots = xs[slot_tok_c]                        # [CAP, D] f32
    xt = np.ascontiguousarray(
        x_slots.T.reshape(DC, P, CAP).transpose(1, 0, 2))  # [P, DC, CAP]
    xt = _bf16(xt)
    o = 0
    xsegs = {}
    for s, wd in enumerate(widths):
        xsegs[f"xt{s}"] = np.ascontiguousarray(xt[:, :, o:o + wd])
        o += wd
    return {
        **xsegs,
        "gates": slot_gate_c.reshape(1, CAP),
        "wg": np.stack([wg_pe[e] for e in sets_c]),
        "w1": np.stack([w1_pe[e] for e in sets_c]),
        "w2": np.stack([w2_pe[e] for e in sets_c]),
        "bg": np.concatenate([bg_pe[e] for e in sets_c], 1),
        "b1": np.concatenate([b1_pe[e] for e in sets_c], 1),
        "b2": np.concatenate([b2_pe[e] for e in sets_c], 1),
    }


def prepare(x, centroid, Wg, bg, W1, b1, W2, b2):
    """Host side: routing + dispatch. Returns (nc, in_maps, plan)."""
    x = np.asarray(x, np.float32)
    centroid = np.asarray(centroid, np.float32)
    Wg = np.asarray(Wg, np.float32)
    W1 = np.asarray(W1, np.float32)
    W2 = np.asarray(W2, np.float32)
    bg = np.asarray(bg, np.float32)
    b1 = np.asarray(b1, np.float32)
    b2 = np.asarray(b2, np.float32)

    sel, w = _routing(x, centroid)
    widths, sets, slot_tok, slot_gate, slot_exp = _build_plan(sel, w)

    key = ("v3", widths)
    if key not in _prog_cache:
        _prog_cache[key] = _build_program(widths)
    nc = _prog_cache[key]
    global _last_widths
    _last_widths = widths

    WgP = np.zeros((E, D, HP), np.float32)
    WgP[:, :, :H] = Wg
    W1P = np.zeros((E, D, HP), np.float32)
    W1P[:, :, :H] = W1
    W2P = np.zeros((E, HP, D), np.float32)
    W2P[:, :H, :] = W2
    bgP = np.zeros((E, HP), np.float32)
    bgP[:, :H] = bg
    b1P = np.zeros((E, HP), np.float32)
    b1P[:, :H] = b1
    # [h, p, d, c] layouts
    wg_pe = [_bf16(WgP[e].reshape(DC, P, HC, P).transpose(2, 1, 0, 3))
             for e in range(E)]
    w1_pe = [_bf16(W1P[e].reshape(DC, P, HC, P).transpose(2, 1, 0, 3))
             for e in range(E)]
    # [p, h, d, c] layout
    w2_pe = [_bf16(W2P[e].reshape(HC, P, DC, P).transpose(1, 0, 2, 3))
             for e in range(E)]
    bg_pe = [np.ascontiguousarray(bgP[e].reshape(HC, P).T) for e in range(E)]
    b1_pe = [np.ascontiguousarray(b1P[e].reshape(HC, P).T) for e in range(E)]
    b2_pe = [np.ascontiguousarray(b2[e].reshape(DC, P).T) for e in range(E)]

    xs = x.reshape(T, D)
    in_maps = [
        _pack_core_inputs(widths, sets[c], slot_tok[c], slot_gate[c], xs,
                          wg_pe, w1_pe, w2_pe, bg_pe, b1_pe, b2_pe)
        for c in range(N_CORES)
    ]
    plan = (slot_tok, slot_gate, slot_exp)
    return nc, in_maps, plan


def combine(results, plan):
    """Scatter-add per-core outputs back to the full [B, S, D] output."""
    slot_tok, slot_gate, slot_exp = plan
    out = np.zeros((T, D), np.float32)
    cap = len(slot_tok[0])
    for e in range(E):
        idxs, vals = [], []
        for c in range(N_CORES):
            ovals = results[c]["out"]  # [DC, P, CAP] f32
            m = (slot_exp[c] == e) & (slot_gate[c] != 0.0)
            if not m.any():
                continue
            sl = np.nonzero(m)[0]
            idxs.append(slot_tok[c][sl])
            ov = np.asarray(ovals).astype(np.float32)
            vals.append(ov.reshape(D, cap)[:, sl].T)  # [n, D]
        if not idxs:
            continue
        idx = np.concatenate(idxs)
        val = np.concatenate(vals)
        # token indices are unique within one expert
        out[idx] += val
    return out.reshape(B, S, D)


def kernel(x, centroid, Wg, bg, W1, b1, W2, b2):
    import sys
    if "/opt/trn_rl_repo" not in sys.path:
        sys.path.insert(0, "/opt/trn_rl_repo")
    from concourse.bass_utils import run_bass_kernel_spmd

    nc, in_maps, plan = prepare(x, centroid, Wg, bg, W1, b1, W2, b2)
    res = run_bass_kernel_spmd(nc, in_maps, list(range(N_CORES)))
    return combine(res.results, plan)

